# revision 16
# baseline (speedup 1.0000x reference)
"""Bidirectional GRU encoder (nn_BiEncoder) on 8 Trainium2 NeuronCores.

Strategy:
 - The L-sequential GRU recurrence cannot be split across cores (collective
   floor ~5us/step x 512 steps is prohibitive), so each direction's
   recurrence runs fully core-local: cores 0-3 handle the ltr direction,
   cores 4-7 the rtl direction (same SPMD program, different input data),
   with batch sharded 16 sequences per core.
 - Per core: input projections x @ W_{r,z,h}.T + b are computed on-device
   into SBUF (interleaved with the recurrence as PE gap-filler), and the
   recurrence keeps the hidden state both row-major (16, 1024) for pointwise
   ops and transposed (1024, 16) bf16 as the matmul stationary operand
   (maintained via PE transposes).
 - Matmul operands are bf16 (PE streams 1 column/cycle; fp32 would be 4x
   slower), accumulation and pointwise math fp32.
 - Ragged-length handling is exact and done on host: padded positions are
   zeroed after the fact, and the recurrence runs unmasked on the device --
   frozen state never feeds any valid output (outputs at t >= length are
   zeroed; `last` reads t = length-1), and sequences never mix.
 - Host also does: embedding gather, rtl flips, output assembly, dec_init.
"""

import os
import sys

import numpy as np

sys.path.insert(0, "/opt/trn_rl_repo")

import ml_dtypes

BF16 = ml_dtypes.bfloat16

V, E, H, L_FULL, B = 32000, 512, 1024, 512, 64
NCORES = 8
BS = B // 4  # 16 sequences per core; 4 cores per direction


# ---------------------------------------------------------------------------
# Walrus workaround: this compiler build rejects >1 sync-wait per instruction
# ("Too many sync wait commands"). Split excess waits onto single-wait drain
# carrier instructions inserted just before, on the same engine.
# ---------------------------------------------------------------------------

def _install_wait_splitter(tile):
    if getattr(tile.TileContext, "_wait_splitter_installed", False):
        return
    orig = tile.TileContext._drain_and_barrier

    def patched(self, tick_clock, wait_clock):
        nc = self.nc
        orig(self, tick_clock, wait_clock)
        eng_by_val = None
        for bb in list(nc.m.functions[0].blocks):
            i = 0
            insts = bb.instructions
            while i < len(insts):
                inst = insts[i]
                si = inst.sync_info
                if si is not None and si.on_wait and len(si.on_wait) > 1:
                    if eng_by_val is None:
                        eng_by_val = {
                            "PE": nc.tensor,
                            "DVE": nc.vector,
                            "Activation": nc.scalar,
                            "Pool": nc.gpsimd,
                            "SP": nc.sync,
                        }
                    waits = list(si.on_wait)
                    si.on_wait = [waits[0]]
                    cur_bb = nc.cur_bb.bb
                    for j, w in enumerate(waits[1:]):
                        eng = eng_by_val.get(inst.engine.value, nc.sync)
                        carrier = eng.nop(nofuse=True)
                        csi = carrier.ins.sync_info
                        if csi is None:
                            carrier.ins.sync_info = si.__replace__(
                                on_wait=[w], on_update=[]
                            )
                        else:
                            csi.on_wait = [w]
                            csi.on_update = []
                        assert cur_bb.instructions[-1].name == carrier.ins.name
                        cur_bb.instructions.pop()
                        bb.instructions.insert(i + j, carrier.ins)
                    i += len(waits) - 1
                i += 1

    tile.TileContext._drain_and_barrier = patched
    tile.TileContext._wait_splitter_installed = True


# ---------------------------------------------------------------------------
# Device program (identical on all 8 cores)
# ---------------------------------------------------------------------------

def _build_nc(L):
    import concourse.bass as bass
    import concourse.mybir as mybir
    import concourse.tile as tile
    from concourse.masks import make_identity
    from contextlib import ExitStack

    _install_wait_splitter(tile)

    f32 = mybir.dt.float32
    bf16 = mybir.dt.bfloat16
    Sig = mybir.ActivationFunctionType.Sigmoid
    Tanh = mybir.ActivationFunctionType.Tanh

    MT = (L * BS) // 128  # projection M-tiles (8 steps each)
    KT_E = E // 128  # 4
    KT_H = H // 128  # 8
    G = 3 * H  # 3072 gate columns, order [r | z | h]

    nc = bass.Bass()
    xT_d = nc.dram_tensor("xT", [MT, KT_E, 128, 128], bf16, kind="ExternalInput")
    uT_d = nc.dram_tensor("uT", [KT_H, 128, G], bf16, kind="ExternalInput")
    wT_d = nc.dram_tensor("wT", [KT_E, 128, G], bf16, kind="ExternalInput")
    bias_d = nc.dram_tensor("bias", [1, G], f32, kind="ExternalInput")
    id4_d = nc.dram_tensor("ident4", [128, 16], f32, kind="ExternalInput")
    hout = nc.dram_tensor("hout", [L, BS, H], f32, kind="ExternalOutput")

    with ExitStack() as ctx:
        tc = ctx.enter_context(tile.TileContext(nc))
        singles = ctx.enter_context(tc.tile_pool(name="singles", bufs=1))
        xt_pool = ctx.enter_context(tc.tile_pool(name="xt", bufs=8))
        projsb_pool = ctx.enter_context(tc.tile_pool(name="projsb", bufs=3))
        pp_pool = ctx.enter_context(tc.tile_pool(name="pp", bufs=2, space="PSUM"))
        ps_rh_pool = ctx.enter_context(tc.tile_pool(name="ps_rh", bufs=2, space="PSUM"))
        ps_t_pool = ctx.enter_context(tc.tile_pool(name="ps_t", bufs=1, space="PSUM"))
        gates = ctx.enter_context(tc.tile_pool(name="gates", bufs=1))
        xp_pool = ctx.enter_context(tc.tile_pool(name="xp", bufs=2))
        st_pool = ctx.enter_context(tc.tile_pool(name="st", bufs=2))

        # --- resident weights ---
        uT = singles.tile([128, KT_H, G], bf16, tag="uT")
        for k in range(KT_H):
            nc.sync.dma_start(out=uT[:, k, :], in_=uT_d[k])
        wT = singles.tile([128, KT_E, G], bf16, tag="wT")
        for k in range(KT_E):
            nc.sync.dma_start(out=wT[:, k, :], in_=wT_d[k])
        biasb = singles.tile([128, G], f32, tag="biasb")
        bap = bias_d[:, :]
        nc.sync.dma_start(
            out=biasb,
            in_=bass.AP(tensor=bap.tensor, offset=bap.offset, ap=[[0, 128], bap.ap[-1]]),
        )
        ident4 = singles.tile([128, 16], f32, tag="id4")
        nc.sync.dma_start(out=ident4, in_=id4_d[:, :])
        ident16 = singles.tile([16, 16], bf16, tag="id16")
        make_identity(nc, ident16)

        proj_tiles = {}

        def emit_proj(m):
            xts = []
            for k in range(KT_E):
                xt = xt_pool.tile([128, 128], bf16, name=f"xt{m}_{k}", tag="xt")
                nc.sync.dma_start(out=xt, in_=xT_d[m, k])
                xts.append(xt)
            psb = projsb_pool.tile([128, G], bf16, name=f"psb{m}", tag="projsb")
            # deprioritized: pure PE/DVE gap-filler behind the recurrence chain
            with tc.high_priority(offset=-(1 << 24)):
                for n in range(G // 512):
                    pp = pp_pool.tile([128, 512], f32, name=f"pp{m}_{n}", tag="pp")
                    for k in range(KT_E):
                        nc.tensor.matmul(
                            pp,
                            xts[k],
                            wT[:, k, n * 512 : (n + 1) * 512],
                            start=(k == 0),
                            stop=(k == KT_E - 1),
                        )
                    nc.vector.tensor_add(
                        psb[:, n * 512 : (n + 1) * 512],
                        pp,
                        biasb[:, n * 512 : (n + 1) * 512],
                    )
            proj_tiles[m] = psb

        # --- initial state ---
        hB = st_pool.tile([128, 512], f32, tag="hB")
        nc.vector.memset(hB, 0.0)
        hT16 = st_pool.tile([128, KT_H, BS], bf16, tag="hT16")
        nc.vector.memset(hT16, 0.0)
        hT32 = st_pool.tile([128, KT_H, BS], f32, tag="hT32")
        nc.vector.memset(hT32, 0.0)

        LOOKAHEAD = 2
        for m in range(min(LOOKAHEAD, MT)):
            emit_proj(m)

        for t in range(L):
            m, row = divmod(t, 128 // BS)
            if row == 0 and m + LOOKAHEAD < MT:
                emit_proj(m + LOOKAHEAD)
            # row slice of the projection tile, moved to partition base 0
            xp = xp_pool.tile([BS, G], bf16, name=f"xp{t}", tag="xp")
            nc.sync.dma_start(out=xp, in_=proj_tiles[m][row * BS : (row + 1) * BS, :])

            # MM1: r0/r1/z0/z1 column-packed into one PSUM bank at partition
            # bases 0/32/64/96 -- four streams run concurrently in different
            # 32-column groups of the PE array.
            psA = ps_rh_pool.tile([128, 512], f32, name=f"psA{t}", tag="ps")
            outs1 = [psA[32 * c : 32 * c + BS, :] for c in range(4)]
            for c in range(4):
                nc.tensor.matmul(outs1[c], ident16, xp[:, c * 512 : (c + 1) * 512],
                                 start=True, stop=False, tile_position=(0, 32 * c),
                                 skip_group_check=True)
            for k in range(KT_H):
                for c in range(4):
                    nc.tensor.matmul(
                        outs1[c], hT16[:, k, :], uT[:, k, c * 512 : (c + 1) * 512],
                        start=False, stop=(k == KT_H - 1), tile_position=(0, 32 * c),
                        skip_group_check=True)

            # r path: sigmoid off PSUM (chunk c at partition base 32c),
            # row-packed transposes, then r*h in T-space
            r32 = gates.tile([128, 512], f32, name=f"r32{t}", tag="r32")
            rT = ps_t_pool.tile([128, KT_H, BS], f32, name=f"rT{t}", tag="rT")
            rhT = st_pool.tile([128, KT_H, BS], bf16, name=f"rhT{t}", tag="rhT")
            for c in range(2):
                base = 32 * c
                nc.scalar.activation(r32[base : base + BS, :], outs1[c], Sig)
                for j in range(4):
                    k = 4 * c + j
                    nc.tensor.transpose(
                        rT[:, k, :], r32[base : base + BS, j * 128 : (j + 1) * 128],
                        ident4[base : base + BS, :], tile_position=(base, 0))
                nc.vector.tensor_mul(
                    rhT[:, 4 * c : 4 * c + 4, :],
                    rT[:, 4 * c : 4 * c + 4, :],
                    hT32[:, 4 * c : 4 * c + 4, :])

            # MM2: h0/h1 column-packed at bases 64/96 (aligned with z chunks)
            psB = ps_rh_pool.tile([128, 512], f32, name=f"psB{t}", tag="ps")
            outs2 = [psB[64 + 32 * c : 64 + 32 * c + BS, :] for c in range(2)]
            for c in range(2):
                nc.tensor.matmul(outs2[c], ident16,
                                 xp[:, 2 * H + c * 512 : 2 * H + (c + 1) * 512],
                                 start=True, stop=False, tile_position=(0, 64 + 32 * c),
                                 skip_group_check=True)
            for k in range(KT_H):
                for c in range(2):
                    nc.tensor.matmul(
                        outs2[c], rhT[:, k, :],
                        uT[:, k, 2 * H + c * 512 : 2 * H + (c + 1) * 512],
                        start=False, stop=(k == KT_H - 1),
                        tile_position=(0, 64 + 32 * c), skip_group_check=True)

            # z path (bases 64/96, aligned with the h chunks for the combine);
            # w = (1-z)*hB precomputed off the critical chain during MM2
            z = gates.tile([128, 512], f32, name=f"z{t}", tag="z")
            w = gates.tile([128, 512], f32, name=f"w{t}", tag="w")
            for c in range(2):
                base = 64 + 32 * c
                psl = slice(base, base + BS)
                nc.scalar.activation(z[psl, :], outs1[2 + c], Sig)
                nc.vector.tensor_mul(w[psl, :], z[psl, :], hB[psl, :])
                nc.vector.tensor_sub(w[psl, :], hB[psl, :], w[psl, :])

            # candidate + combine + state transposes, 256-col pipelined tail
            hp = gates.tile([128, 512], f32, name=f"hp{t}", tag="hp")
            hB2 = st_pool.tile([128, 512], f32, name=f"hB{t}", tag="hB")
            hT = ps_t_pool.tile([128, KT_H, BS], f32, name=f"hT{t}", tag="hT")
            hT16n = st_pool.tile([128, KT_H, BS], bf16, name=f"hT16_{t}", tag="hT16")
            hT32n = st_pool.tile([128, KT_H, BS], f32, name=f"hT32_{t}", tag="hT32")
            for c in range(2):
                base = 64 + 32 * c
                psl = slice(base, base + BS)
                nc.scalar.activation(hp[psl, :], outs2[c], Tanh)
                nc.vector.tensor_mul(hp[psl, :], z[psl, :], hp[psl, :])
                nc.vector.tensor_add(hB2[psl, :], w[psl, :], hp[psl, :])
                for j in range(4):
                    k = 4 * c + j
                    nc.tensor.transpose(
                        hT[:, k, :], hB2[psl, j * 128 : (j + 1) * 128],
                        ident4[psl, :], tile_position=(base, 0))
                hsl = slice(4 * c, 4 * c + 4)
                nc.vector.tensor_copy(hT16n[:, hsl, :], hT[:, hsl, :])
                nc.scalar.copy(hT32n[:, hsl, :], hT[:, hsl, :])
            for c in range(2):
                base = 64 + 32 * c
                nc.sync.dma_start(out=hout[t][:, c * 512 : (c + 1) * 512],
                                  in_=hB2[base : base + BS, :])
            hB, hT16, hT32 = hB2, hT16n, hT32n

    return nc


# ---------------------------------------------------------------------------
# Host-side helpers
# ---------------------------------------------------------------------------

def _flip_padded(h, lengths):
    Lm = h.shape[0]
    t = np.arange(Lm)[:, None]
    idx = lengths[None, :].astype(np.int64) - 1 - t
    valid = idx >= 0
    idxc = np.maximum(idx, 0)
    if h.ndim == 3:
        g = np.take_along_axis(h, idxc[:, :, None], axis=0)
        return np.where(valid[:, :, None], g, np.zeros((), h.dtype))
    g = np.take_along_axis(h, idxc, axis=0)
    return np.where(valid, g, np.zeros((), h.dtype))


def _prep_core_inputs(x_slice, W_all, U_all, b_all, L):
    """x_slice: (L, BS, E) f32. Returns the per-core input dict."""
    MT = (L * BS) // 128
    xr = np.ascontiguousarray(x_slice).reshape(L * BS, E)
    A = xr.reshape(MT, 128, E // 128, 128)  # [m, j, k, i]
    xT = np.ascontiguousarray(A.transpose(0, 2, 3, 1)).astype(BF16)
    wT = np.ascontiguousarray(W_all.T.reshape(E // 128, 128, 3 * H)).astype(BF16)
    uT = np.ascontiguousarray(U_all.T.reshape(H // 128, 128, 3 * H)).astype(BF16)
    bias = np.ascontiguousarray(b_all.reshape(1, 3 * H)).astype(np.float32)
    id4 = np.zeros((128, 16), np.float32)
    for c in range(4):
        id4[32 * c : 32 * c + 16] = np.eye(16, dtype=np.float32)
    return {"xT": xT, "uT": uT, "wT": wT, "bias": bias, "ident4": id4}


_NC_CACHE = {}


def _get_nc(L):
    if L not in _NC_CACHE:
        _NC_CACHE[L] = _build_nc(L)
    return _NC_CACHE[L]


def _run_cores(in_maps, L, trace=False):
    from concourse.bass_utils import run_bass_kernel_spmd

    nc = _get_nc(L)
    return run_bass_kernel_spmd(nc, in_maps, core_ids=list(range(NCORES)), trace=trace)


def kernel(emb,
           ltr_Wh, ltr_Wz, ltr_Wr, ltr_Uh, ltr_Uz, ltr_Ur, ltr_bh, ltr_bz, ltr_br,
           rtl_Wh, rtl_Wz, rtl_Wr, rtl_Uh, rtl_Uz, rtl_Ur, rtl_bh, rtl_bz, rtl_br,
           Ws_w, Ws_b, padded_tokens, lengths, _trace=False, _L=None):
    L = _L or L_FULL
    emb = np.asarray(emb, np.float32)
    tokens = np.asarray(padded_tokens).astype(np.int64)[:L]
    lengths = np.asarray(lengths).astype(np.int32)
    lengths_c = np.minimum(lengths, L)

    tok_rev = _flip_padded(tokens, lengths_c).astype(np.int64)

    packs = {}
    for pre, Wh, Wz, Wr, Uh, Uz, Ur, bh, bz, br in (
        ("ltr", ltr_Wh, ltr_Wz, ltr_Wr, ltr_Uh, ltr_Uz, ltr_Ur, ltr_bh, ltr_bz, ltr_br),
        ("rtl", rtl_Wh, rtl_Wz, rtl_Wr, rtl_Uh, rtl_Uz, rtl_Ur, rtl_bh, rtl_bz, rtl_br),
    ):
        W_all = np.concatenate(
            [np.asarray(Wr, np.float32), np.asarray(Wz, np.float32),
             np.asarray(Wh, np.float32)], axis=0)
        U_all = np.concatenate(
            [np.asarray(Ur, np.float32), np.asarray(Uz, np.float32),
             np.asarray(Uh, np.float32)], axis=0)
        b_all = np.concatenate(
            [np.asarray(br, np.float32), np.asarray(bz, np.float32),
             np.asarray(bh, np.float32)], axis=0)
        packs[pre] = (W_all, U_all, b_all)

    x_ltr = emb[tokens]        # (L, B, E)
    x_rtl = emb[tok_rev]

    in_maps = []
    for c in range(NCORES):
        d = "ltr" if c < 4 else "rtl"
        b0 = (c % 4) * BS
        x = x_ltr if d == "ltr" else x_rtl
        W_all, U_all, b_all = packs[d]
        in_maps.append(_prep_core_inputs(x[:, b0 : b0 + BS, :], W_all, U_all, b_all, L))

    res = _run_cores(in_maps, L, trace=_trace)

    h_ltr = np.concatenate([res.results[c]["hout"] for c in range(4)], axis=1)
    h_rtl_rev = np.concatenate([res.results[c]["hout"] for c in range(4, 8)], axis=1)
    h_rtl = _flip_padded(h_rtl_rev, lengths_c)

    mask = (np.arange(L)[:, None] < lengths_c[None, :]).astype(np.float32)
    hiddens = np.concatenate([h_ltr, h_rtl], axis=2) * mask[:, :, None]

    last = h_ltr[np.maximum(lengths_c, 1) - 1, np.arange(B)]  # (B, H)
    dec_init = last @ np.asarray(Ws_w, np.float32).T + np.asarray(Ws_b, np.float32)

    if _trace:
        kernel.last_result = res
    return hiddens.astype(np.float32), dec_init.astype(np.float32)
